# revision 32
# baseline (speedup 1.0000x reference)
"""Causal self-attention with RoPE on 8 Trainium2 NeuronCores.

Problem: B=2, S=2048, H=16 heads, D=128, HID=2048, fp32.
  qkv = x @ w_qkv.T ; RoPE(q, k) ; causal softmax(q k^T / sqrt(D)) @ v ; out @ w_o.T

Sharding (hardcoded): core c in 0..7 handles batch b = c // 4 and head group
g = c % 4 (heads 4g..4g+4). Each core computes a partial (S, HID) output
contracted over its 512 hidden dims of the o-projection; the host sums the 4
bf16 partials per batch.

All matmul operands are bf16: fp32/fp32r LDWEIGHTS take ~220ns and serialize
with the matmuls (LDW:MM emission is 1:1), while bf16 gets Fast Weight Load
(~53-97ns, hidden) and halves SBUF/DMA/DVE cost. PSUM accumulation stays
fp32; end-to-end rel err ~3e-3 vs the 2e-2 gate.

Schedule: the engine queues are strict FIFO, so all cross-engine overlap is
arranged by emission order (software pipelining):
 - Attention for an (h, si-block) unit is a per-chunk 3-stage pipeline
   (scores matmul -> ACT exp -> l/o matmuls) with lookahead 2, so the PE
   never waits on ACT at the queue head.
 - Attention units for si-blocks 0-1 depend only on the first s-half, and
   are dribbled into the second half's projection chains (generator steps
   between projection matmuls): ACT exp runs under projection matmuls.
 - The o-projection (pure PE work) for retired si-blocks is dribbled into
   the remaining attention units' stall slots the same way.
Softmax: scores^T orientation (denominator = ones-vector matmul), no
max-subtraction (scores are O(5); exp is safe in fp32), causal masking via
an additive -1e30 triangle on diagonal 128x128 blocks + column-restricted
matmuls. The attention output o^T[d, si] is exactly the o-projection's lhsT.
"""

from collections import deque
import os

import numpy as np
import ml_dtypes

import concourse.bacc as bacc
import concourse.tile as tile
from concourse import mybir
from concourse.bass_utils import run_bass_kernel_spmd

B, S, H, D = 2, 2048, 16, 128
HID = H * D
THETA = 10000.0
SCALE = 1.0 / float(np.sqrt(D))
NH = 4                 # heads per core
NC = 8                 # cores
NKC = HID // 128       # contraction chunks for qkv projection
SB = 512               # matmul moving free dim
NSB = S // SB          # si blocks
SH = S // 2            # s-half
F32 = mybir.dt.float32
BF16 = mybir.dt.bfloat16

LAST_RESULT = None  # BassKernelResults of the most recent run (for test harness)
MM_MODE = "bf16"


def _build_nc():
    nc = bacc.Bacc("TRN2", target_bir_lowering=False, debug=False, num_devices=NC)

    xT = nc.dram_tensor("xT", [HID, S], BF16, kind="ExternalInput")
    wqkvT = nc.dram_tensor("wqkvT", [HID, 3 * NH * 128], BF16, kind="ExternalInput")
    woT = nc.dram_tensor("woT", [NH * 128, HID], BF16, kind="ExternalInput")
    cosT = nc.dram_tensor("cosT", [D, S], BF16, kind="ExternalInput")
    sinST = nc.dram_tensor("sinST", [D, S], BF16, kind="ExternalInput")
    maskadd = nc.dram_tensor("maskadd", [128, 128], F32, kind="ExternalInput")
    out = nc.dram_tensor("out", [S, HID], BF16, kind="ExternalOutput")

    with tile.TileContext(nc) as tc:
        with tc.tile_pool(name="pmisc", bufs=1) as pmisc, \
             tc.tile_pool(name="pw", bufs=1) as pw, \
             tc.tile_pool(name="pqk", bufs=1) as pqk, \
             tc.tile_pool(name="pvn", bufs=1) as pvn, \
             tc.tile_pool(name="pwo", bufs=1) as pwo, \
             tc.tile_pool(name="pfin", bufs=2) as pfin:
            ones_f32 = pmisc.tile([128, 1], F32, name="ones_f32")
            nc.vector.memset(ones_f32, 1.0)
            ones_t = pmisc.tile([128, 1], BF16, name="ones")
            nc.vector.tensor_copy(ones_t, ones_f32)
            # tri on the gpsimd queue so the scalar queue's first DMAs are
            # the weight chunks the first matmuls wait on
            tri_t = pmisc.tile([128, 128], F32, name="tri")
            nc.gpsimd.dma_start(out=tri_t, in_=maskadd[:, :])

            # warm-up: dummy matmuls keep the PE busy while the first x/w
            # DMAs land, so HAM un-throttles (K=8/8) before real work and
            # the ramp isn't paid on the first projection chain
            wsrc = pmisc.tile([128, SB], BF16, name="wsrc")
            nc.gpsimd.memset(wsrc, 0.0)
            with tc.tile_pool(name="pwarm", bufs=1, space="PSUM") as pwarm:
                warm_ps = pwarm.tile([1, SB], F32, name="warm")
                for _ in range(26):
                    nc.tensor.matmul(warm_ps, ones_t, wsrc,
                                     start=True, stop=True)

            # full qkv weights resident in SBUF, one tile per contraction
            # chunk so the first matmuls only wait on their own chunk's DMA
            w_kc = []
            for kc in range(NKC):
                wt = pw.tile([128, 3 * NH * 128], BF16, name=f"w_{kc}")
                nc.scalar.dma_start(
                    out=wt, in_=wqkvT[kc * 128:(kc + 1) * 128, :])
                w_kc.append(wt)

            qT = [pqk.tile([128, S], BF16, name=f"qT_{h}") for h in range(NH)]
            kT = [pqk.tile([128, S], BF16, name=f"kT_{h}") for h in range(NH)]
            # natural-layout v, all heads fused: vn[g][sp, j, h*128+d] covers
            # s-chunks 4g+j
            vn = [pvn.tile([128, 4, NH * 128], BF16, name=f"vn_{g}")
                  for g in range(4)]
            # attention output aliases qT: q columns for an si-block are dead
            # once that block's scores are done, and the normalized output is
            # written only after that point.
            outT = qT
            wo = [pwo.tile([128, HID], BF16, name=f"wo_{h}")
                  for h in range(NH)]

            # ---- Phase A body: q/k/v projection + RoPE for one s-half ----
            # bfill dribbles attention pipeline steps between the chains.
            def emit_half(px, ptrig, psh, half, ppa_bufs, bfill, nest_psum):
                s0 = half * SH
                xh = []
                for kc in range(NKC):
                    xt = px.tile([128, SH], BF16, name=f"xh_{kc}")
                    nc.sync.dma_start(
                        out=xt, in_=xT[kc * 128:(kc + 1) * 128, s0:s0 + SH])
                    xh.append(xt)
                cos_t = ptrig.tile([D, SH], BF16, name="cosT")
                sin_t = ptrig.tile([D, SH], BF16, name="sinST")
                nc.gpsimd.dma_start(out=cos_t, in_=cosT[:, s0:s0 + SH])
                nc.gpsimd.dma_start(out=sin_t, in_=sinST[:, s0:s0 + SH])

                def qk_part(ppa):
                    # q/k projection, transposed output [d, s]; kc outer so
                    # the two si-blocks of one (h,kind) share a weight tile
                    for h in range(NH):
                        for kind, dst in ((0, qT[h]), (1, kT[h])):
                            ot = kind * NH + h
                            ps = [ppa.tile([128, SB], F32, name=f"qkps{sb}")
                                  for sb in range(SH // SB)]
                            for kc in range(NKC):
                                wt = w_kc[kc][:, ot * 128:(ot + 1) * 128]
                                for sb_i in range(SH // SB):
                                    nc.tensor.matmul(
                                        ps[sb_i], wt,
                                        xh[kc][:, sb_i * SB:(sb_i + 1) * SB],
                                        start=(kc == 0), stop=(kc == NKC - 1))
                                if kc % 3 == 2:
                                    bfill(1)
                            for sb_i in range(SH // SB):
                                lo = s0 + sb_i * SB
                                nc.any.tensor_copy(dst[:, lo:lo + SB],
                                                   ps[sb_i])
                        # RoPE for this head's half, in place (hides under
                        # the remaining projection matmuls)
                        for t in (qT[h], kT[h]):
                            sh_t = psh.tile([128, SH], BF16, name="shuf")
                            nc.gpsimd.dma_start(out=sh_t[0:64, :],
                                                in_=t[64:128, s0:s0 + SH])
                            nc.gpsimd.dma_start(out=sh_t[64:128, :],
                                                in_=t[0:64, s0:s0 + SH])
                            nc.vector.tensor_mul(sh_t, sh_t, sin_t)
                            nc.vector.tensor_mul(t[:, s0:s0 + SH],
                                                 t[:, s0:s0 + SH], cos_t)
                            nc.vector.tensor_add(t[:, s0:s0 + SH],
                                                 t[:, s0:s0 + SH], sh_t)

                def v_part(pvp):
                    # v projection, natural layout [s, 4 heads x d];
                    # st-outer so each chunk's PSUM->SBUF copy overlaps the
                    # next chunk's matmuls instead of serializing at the end
                    for st in range(8):
                        vps = pvp.tile([128, NH * 128], F32, name="vps")
                        for kc in range(NKC):
                            nc.tensor.matmul(
                                vps,
                                xh[kc][:, st * 128:(st + 1) * 128],
                                w_kc[kc][:, 2 * NH * 128:],
                                start=(kc == 0), stop=(kc == NKC - 1))
                            if kc % 4 == 3:
                                bfill(1)
                        sg = half * 8 + st   # global s-chunk
                        nc.any.tensor_copy(vn[sg // 4][:, sg % 4, :], vps)

                if nest_psum:
                    # distinct banks for qk and v pools: the v matmuls don't
                    # wait for the qk copies to drain the qk banks
                    with tc.tile_pool(name="ppa", bufs=ppa_bufs,
                                      space="PSUM") as ppa, \
                         tc.tile_pool(name="pvp", bufs=2,
                                      space="PSUM") as pvp:
                        qk_part(ppa)
                        v_part(pvp)
                else:
                    with tc.tile_pool(name="ppa", bufs=ppa_bufs,
                                      space="PSUM") as ppa:
                        qk_part(ppa)
                    with tc.tile_pool(name="pvp", bufs=2,
                                      space="PSUM") as pvp:
                        v_part(pvp)

            def no_fill(n):
                pass

            with tc.tile_pool(name="px", bufs=2) as px, \
                 tc.tile_pool(name="ptrig", bufs=1) as ptrig, \
                 tc.tile_pool(name="psh", bufs=2) as psh:

                # first half: plain, full PSUM for the projections
                emit_half(px, ptrig, psh, 0, 2, no_fill, nest_psum=True)

                # attention pools stay open from here through main B
                with tc.tile_pool(name="pexp", bufs=6) as pexp, \
                     tc.tile_pool(name="prr", bufs=2) as prr, \
                     tc.tile_pool(name="psc", bufs=3, space="PSUM") as psc, \
                     tc.tile_pool(name="plp", bufs=1, space="PSUM") as plp, \
                     tc.tile_pool(name="pop", bufs=1, space="PSUM") as pop:

                    for h in range(NH):
                        nc.scalar.dma_start(
                            out=wo[h], in_=woT[h * 128:(h + 1) * 128, :])

                    def b_unit_steps(h, sib):
                        """Attention unit (h, si-block): per-chunk 3-stage
                        software pipeline, yielding between stages."""
                        si0 = sib * SB
                        nch = 4 * (sib + 1)

                        def sc(c):
                            dg = c - (nch - 4)    # diagonal offset
                            lo = dg * 128 if dg > 0 else 0
                            s_ps = psc.tile([128, SB], F32, name="sps")
                            e_t = pexp.tile([128, SB], BF16, name="exp")
                            nc.tensor.matmul(
                                s_ps[:, lo:],
                                kT[h][:, c * 128:(c + 1) * 128],
                                qT[h][:, si0 + lo:si0 + SB],
                                start=True, stop=True)
                            if dg >= 0:
                                nc.vector.tensor_add(
                                    s_ps[:, lo:lo + 128],
                                    s_ps[:, lo:lo + 128], tri_t)
                            return (c, lo, s_ps, e_t)

                        def ex(stt):
                            c, lo, s_ps, e_t = stt
                            nc.scalar.activation(
                                out=e_t[:, lo:], in_=s_ps[:, lo:],
                                func=mybir.ActivationFunctionType.Exp,
                                scale=SCALE)

                        def lo_mms(stt):
                            # o before l: the o-matmul's 128-col weight load
                            # pull-ahead works; the 1-col ones load doesn't
                            c, lo, s_ps, e_t = stt
                            nc.tensor.matmul(
                                o_ps[:, lo:],
                                vn[c // 4][:, c % 4, h * 128:(h + 1) * 128],
                                e_t[:, lo:],
                                start=(c == 0), stop=(c == nch - 1))
                            nc.tensor.matmul(
                                l_ps[:, lo:], ones_t, e_t[:, lo:],
                                start=(c == 0), stop=(c == nch - 1))

                        l_ps = plp.tile([1, SB], F32, name="lps")
                        o_ps = pop.tile([128, SB], F32, name="ops")
                        q = deque()
                        q.append(sc(0))
                        yield
                        ex(q[0])
                        q.append(sc(1))
                        yield
                        for c in range(nch):
                            lo_mms(q.popleft())
                            if c + 1 < nch:
                                ex(q[0])
                            if c + 2 < nch:
                                q.append(sc(c + 2))
                            yield
                        recip = prr.tile([1, SB], F32, name="recip")
                        nc.vector.reciprocal_approx_fast(out=recip, in_=l_ps)
                        rb = prr.tile([128, SB], F32, name="rb")
                        nc.gpsimd.partition_broadcast(rb, recip)
                        nc.vector.tensor_mul(
                            outT[h][:, si0:si0 + SB], o_ps, rb)
                        yield

                    # second half with si-blocks 0-1 attention dribbled in
                    def b_units_01():
                        for sib in (0, 1):
                            for h in range(NH):
                                yield from b_unit_steps(h, sib)

                    bgen = b_units_01()

                    def bfill(n):
                        nonlocal bgen
                        for _ in range(n):
                            if bgen is None:
                                return
                            try:
                                next(bgen)
                            except StopIteration:
                                bgen = None
                                return

                    emit_half(px, ptrig, psh, 1, 1, bfill, nest_psum=False)
                    while bgen is not None:
                        bfill(1)

                    # ---- main attention (si-blocks 2-3) with o-projection
                    # filler for retired si-blocks ----
                    with tc.tile_pool(name="pfc", bufs=2,
                                      space="PSUM") as pfc:
                        c_queue = deque(range(8))   # st chunks of sibs 0-1
                        filler = None

                        def c_steps(st):
                            """O-projection for s-chunk st, yielding after
                            every matmul so it can be dribbled into the
                            attention pipeline."""
                            fin = pfin.tile([128, HID], BF16, name="fin")
                            for ob in range(HID // SB):
                                fps = pfc.tile([128, SB], F32, name="fps")
                                for h in range(NH):
                                    nc.tensor.matmul(
                                        fps,
                                        outT[h][:, st * 128:(st + 1) * 128],
                                        wo[h][:, ob * SB:(ob + 1) * SB],
                                        start=(h == 0), stop=(h == NH - 1))
                                    yield
                                nc.any.tensor_copy(
                                    fin[:, ob * SB:(ob + 1) * SB], fps)
                                nc.sync.dma_start(
                                    out=out[st * 128:(st + 1) * 128,
                                            ob * SB:(ob + 1) * SB],
                                    in_=fin[:, ob * SB:(ob + 1) * SB])

                        def fill(n):
                            nonlocal filler
                            for _ in range(n):
                                if filler is None:
                                    if not c_queue:
                                        return
                                    filler = c_steps(c_queue.popleft())
                                try:
                                    next(filler)
                                except StopIteration:
                                    filler = None

                        for sib in (2, 3):
                            for h in range(NH):
                                for _ in b_unit_steps(h, sib):
                                    fill(2)
                                fill(8)
                            if sib == 2:
                                c_queue.extend(range(8, 12))
                        # tail: the last si-block's four chunks, drained
                        # densely through the same double-buffered pool (no
                        # extra PSUM scope barrier)
                        c_queue.extend(range(4 * (NSB - 1), S // 128))
                        fill(10 ** 6)

    nc.compile()
    return nc


_NC_CACHE = None


def _get_nc():
    global _NC_CACHE
    if _NC_CACHE is None:
        _NC_CACHE = _build_nc()
    return _NC_CACHE


def _host_inputs(x, w_qkv, w_o):
    """Per-core input maps (sharding + layout prep on host)."""
    bf = ml_dtypes.bfloat16
    inv_freq = 1.0 / (THETA ** (np.arange(0, D, 2, dtype=np.float64) / D))
    pos = np.arange(S, dtype=np.float64)
    freqs = pos[:, None] * inv_freq[None, :]          # (S, D/2)
    emb = np.concatenate([freqs, freqs], axis=-1)     # (S, D)
    cosT = np.ascontiguousarray(np.cos(emb).T.astype(bf))           # (D, S)
    sign = np.concatenate([-np.ones(D // 2), np.ones(D // 2)])
    sinST = np.ascontiguousarray((sign[None, :] * np.sin(emb)).T
                                 .astype(bf))                       # (D, S)
    # additive causal triangle for a diagonal 128x128 block of scores^T:
    # keep (add 0) when sj_local <= si_local, else -1e30
    p = np.arange(128)[:, None]
    f = np.arange(128)[None, :]
    maskadd = np.where(p <= f, 0.0, -1e30).astype(np.float32)       # (128, 128)

    xTb = [np.ascontiguousarray(x[b].T.astype(bf)) for b in range(B)]
    in_maps = []
    for c in range(NC):
        b, g = c // 4, c % 4
        rows = slice(g * NH * D, (g + 1) * NH * D)
        wq = w_qkv[0 * HID:1 * HID][rows]
        wk = w_qkv[1 * HID:2 * HID][rows]
        wv = w_qkv[2 * HID:3 * HID][rows]
        wqkvT = np.ascontiguousarray(
            np.concatenate([wq, wk, wv], axis=0).T.astype(bf))      # (HID, 1536)
        woT = np.ascontiguousarray(w_o[:, rows].T.astype(bf))       # (512, HID)
        in_maps.append({
            "xT": xTb[b], "wqkvT": wqkvT, "woT": woT,
            "cosT": cosT, "sinST": sinST, "maskadd": maskadd,
        })
    return in_maps


def kernel(x, w_qkv, w_o):
    global LAST_RESULT
    x = np.asarray(x, dtype=np.float32)
    w_qkv = np.asarray(w_qkv, dtype=np.float32)
    w_o = np.asarray(w_o, dtype=np.float32)

    nc = _get_nc()
    in_maps = _host_inputs(x, w_qkv, w_o)
    trace = bool(int(os.environ.get("BASS_KERNEL_TRACE", "0")))
    last_exc = None
    for _attempt in range(3):
        try:
            res = run_bass_kernel_spmd(
                nc, in_maps, core_ids=list(range(NC)),
                trace=trace, trace_cores=list(range(NC)) if trace else None)
            out_np = [np.asarray(res.results[c]["out"], dtype=np.float32)
                      for c in range(NC)]
            break
        except Exception as e:  # transient NRT device errors: retry
            last_exc = e
    else:
        raise last_exc
    LAST_RESULT = res

    out = np.empty((B, S, HID), dtype=np.float32)
    for b in range(B):
        acc = np.zeros((S, HID), dtype=np.float64)
        for g in range(4):
            acc += out_np[b * 4 + g]
        out[b] = acc.astype(np.float32)
    return out


# revision 35
# speedup vs baseline: 1.0136x; 1.0136x over previous
"""Causal self-attention with RoPE on 8 Trainium2 NeuronCores.

Problem: B=2, S=2048, H=16 heads, D=128, HID=2048, fp32.
  qkv = x @ w_qkv.T ; RoPE(q, k) ; causal softmax(q k^T / sqrt(D)) @ v ; out @ w_o.T

Sharding (hardcoded): core c in 0..7 handles batch b = c // 4 and head group
g = c % 4 (heads 4g..4g+4). Each core computes a partial (S, HID) output
contracted over its 512 hidden dims of the o-projection; the host sums the 4
bf16 partials per batch.

All matmul operands are bf16: fp32/fp32r LDWEIGHTS take ~220ns and serialize
with the matmuls (LDW:MM emission is 1:1), while bf16 gets Fast Weight Load
(~53-97ns, hidden) and halves SBUF/DMA/DVE cost. PSUM accumulation stays
fp32; end-to-end rel err ~3e-3 vs the 2e-2 gate.

Schedule: the engine queues are strict FIFO, so all cross-engine overlap is
arranged by emission order (software pipelining):
 - Attention for an (h, si-block) unit is a per-chunk 3-stage pipeline
   (scores matmul -> ACT exp -> l/o matmuls) with lookahead 2, so the PE
   never waits on ACT at the queue head.
 - Attention units for si-blocks 0-1 depend only on the first s-half, and
   are dribbled into the second half's projection chains (generator steps
   between projection matmuls): ACT exp runs under projection matmuls.
 - The o-projection (pure PE work) for retired si-blocks is dribbled into
   the remaining attention units' stall slots the same way.
Softmax: scores^T orientation (denominator = ones-vector matmul), no
max-subtraction (scores are O(5); exp is safe in fp32), causal masking via
an additive -1e30 triangle on diagonal 128x128 blocks + column-restricted
matmuls. The attention output o^T[d, si] is exactly the o-projection's lhsT.
"""

from collections import deque
import os

import numpy as np
import ml_dtypes

import concourse.bacc as bacc
import concourse.tile as tile
from concourse import mybir
from concourse.bass_utils import run_bass_kernel_spmd

B, S, H, D = 2, 2048, 16, 128
HID = H * D
THETA = 10000.0
SCALE = 1.0 / float(np.sqrt(D))
NH = 4                 # heads per core
NC = 8                 # cores
NKC = HID // 128       # contraction chunks for qkv projection
SB = 512               # matmul moving free dim
NSB = S // SB          # si blocks
SH = S // 2            # s-half
F32 = mybir.dt.float32
BF16 = mybir.dt.bfloat16

LAST_RESULT = None  # BassKernelResults of the most recent run (for test harness)
MM_MODE = "bf16"


def _build_nc():
    nc = bacc.Bacc("TRN2", target_bir_lowering=False, debug=False, num_devices=NC)

    xT = nc.dram_tensor("xT", [HID, S], BF16, kind="ExternalInput")
    wqkvT = nc.dram_tensor("wqkvT", [HID, 3 * NH * 128], BF16, kind="ExternalInput")
    woT = nc.dram_tensor("woT", [NH * 128, HID], BF16, kind="ExternalInput")
    cosT = nc.dram_tensor("cosT", [D, S], BF16, kind="ExternalInput")
    sinST = nc.dram_tensor("sinST", [D, S], BF16, kind="ExternalInput")
    maskadd = nc.dram_tensor("maskadd", [128, 128], F32, kind="ExternalInput")
    out = nc.dram_tensor("out", [S, HID], BF16, kind="ExternalOutput")

    with tile.TileContext(nc) as tc:
        with tc.tile_pool(name="pmisc", bufs=1) as pmisc, \
             tc.tile_pool(name="pw", bufs=1) as pw, \
             tc.tile_pool(name="pqk", bufs=1) as pqk, \
             tc.tile_pool(name="pvn", bufs=1) as pvn, \
             tc.tile_pool(name="pwo", bufs=1) as pwo, \
             tc.tile_pool(name="pfin", bufs=2) as pfin:
            ones_f32 = pmisc.tile([128, 1], F32, name="ones_f32")
            nc.vector.memset(ones_f32, 1.0)
            ones_t = pmisc.tile([128, 1], BF16, name="ones")
            nc.vector.tensor_copy(ones_t, ones_f32)
            # tri on the gpsimd queue so the scalar queue's first DMAs are
            # the weight chunks the first matmuls wait on
            tri_t = pmisc.tile([128, 128], F32, name="tri")
            nc.gpsimd.dma_start(out=tri_t, in_=maskadd[:, :])

            # warm-up: dummy matmuls keep the PE busy while the first x/w
            # DMAs land, so HAM un-throttles (K=8/8) before real work and
            # the ramp isn't paid on the first projection chain
            wsrc = pmisc.tile([128, SB], BF16, name="wsrc")
            nc.gpsimd.memset(wsrc, 0.0)
            with tc.tile_pool(name="pwarm", bufs=1, space="PSUM") as pwarm:
                warm_ps = pwarm.tile([1, SB], F32, name="warm")
                for _ in range(20):
                    nc.tensor.matmul(warm_ps, ones_t, wsrc,
                                     start=True, stop=True)

            # full qkv weights resident in SBUF, one tile per contraction
            # chunk so the first matmuls only wait on their own chunk's DMA
            w_kc = []
            for kc in range(NKC):
                wt = pw.tile([128, 3 * NH * 128], BF16, name=f"w_{kc}")
                nc.scalar.dma_start(
                    out=wt, in_=wqkvT[kc * 128:(kc + 1) * 128, :])
                w_kc.append(wt)

            qT = [pqk.tile([128, S], BF16, name=f"qT_{h}") for h in range(NH)]
            kT = [pqk.tile([128, S], BF16, name=f"kT_{h}") for h in range(NH)]
            # natural-layout v, all heads fused: vn[g][sp, j, h*128+d] covers
            # s-chunks 4g+j
            vn = [pvn.tile([128, 4, NH * 128], BF16, name=f"vn_{g}")
                  for g in range(4)]
            # attention output aliases qT: q columns for an si-block are dead
            # once that block's scores are done, and the normalized output is
            # written only after that point.
            outT = qT
            wo = [pwo.tile([128, HID], BF16, name=f"wo_{h}")
                  for h in range(NH)]

            # ---- Phase A body: q/k/v projection + RoPE for one s-half ----
            # bfill dribbles attention pipeline steps between the chains.
            def emit_half(px, ptrig, psh, half, ppa_bufs, bfill, nest_psum):
                s0 = half * SH
                xh = []
                for kc in range(NKC):
                    xt = px.tile([128, SH], BF16, name=f"xh_{kc}")
                    nc.sync.dma_start(
                        out=xt, in_=xT[kc * 128:(kc + 1) * 128, s0:s0 + SH])
                    xh.append(xt)
                cos_t = ptrig.tile([D, SH], BF16, name="cosT")
                sin_t = ptrig.tile([D, SH], BF16, name="sinST")
                nc.gpsimd.dma_start(out=cos_t, in_=cosT[:, s0:s0 + SH])
                nc.gpsimd.dma_start(out=sin_t, in_=sinST[:, s0:s0 + SH])

                def qk_part(ppa):
                    # q/k projection, transposed output [d, s]; kc outer so
                    # the two si-blocks of one (h,kind) share a weight tile
                    for h in range(NH):
                        for kind, dst in ((0, qT[h]), (1, kT[h])):
                            ot = kind * NH + h
                            ps = [ppa.tile([128, SB], F32, name=f"qkps{sb}")
                                  for sb in range(SH // SB)]
                            for kc in range(NKC):
                                wt = w_kc[kc][:, ot * 128:(ot + 1) * 128]
                                for sb_i in range(SH // SB):
                                    nc.tensor.matmul(
                                        ps[sb_i], wt,
                                        xh[kc][:, sb_i * SB:(sb_i + 1) * SB],
                                        start=(kc == 0), stop=(kc == NKC - 1))
                                if kc % 3 == 2:
                                    bfill(1)
                            for sb_i in range(SH // SB):
                                lo = s0 + sb_i * SB
                                nc.any.tensor_copy(dst[:, lo:lo + SB],
                                                   ps[sb_i])
                        # RoPE for this head's half, in place (hides under
                        # the remaining projection matmuls)
                        for t in (qT[h], kT[h]):
                            sh_t = psh.tile([128, SH], BF16, name="shuf")
                            nc.gpsimd.dma_start(out=sh_t[0:64, :],
                                                in_=t[64:128, s0:s0 + SH])
                            nc.gpsimd.dma_start(out=sh_t[64:128, :],
                                                in_=t[0:64, s0:s0 + SH])
                            nc.vector.tensor_mul(sh_t, sh_t, sin_t)
                            nc.vector.tensor_mul(t[:, s0:s0 + SH],
                                                 t[:, s0:s0 + SH], cos_t)
                            nc.vector.tensor_add(t[:, s0:s0 + SH],
                                                 t[:, s0:s0 + SH], sh_t)

                def v_part(pvp):
                    # v projection, natural layout [s, 4 heads x d];
                    # st-outer so each chunk's PSUM->SBUF copy overlaps the
                    # next chunk's matmuls instead of serializing at the end
                    for st in range(8):
                        vps = pvp.tile([128, NH * 128], F32, name="vps")
                        for kc in range(NKC):
                            nc.tensor.matmul(
                                vps,
                                xh[kc][:, st * 128:(st + 1) * 128],
                                w_kc[kc][:, 2 * NH * 128:],
                                start=(kc == 0), stop=(kc == NKC - 1))
                            if kc % 4 == 3:
                                bfill(1)
                        sg = half * 8 + st   # global s-chunk
                        nc.any.tensor_copy(vn[sg // 4][:, sg % 4, :], vps)

                if nest_psum:
                    # distinct banks for qk and v pools: the v matmuls don't
                    # wait for the qk copies to drain the qk banks
                    with tc.tile_pool(name="ppa", bufs=ppa_bufs,
                                      space="PSUM") as ppa, \
                         tc.tile_pool(name="pvp", bufs=2,
                                      space="PSUM") as pvp:
                        qk_part(ppa)
                        v_part(pvp)
                else:
                    with tc.tile_pool(name="ppa", bufs=ppa_bufs,
                                      space="PSUM") as ppa:
                        qk_part(ppa)
                    with tc.tile_pool(name="pvp", bufs=2,
                                      space="PSUM") as pvp:
                        v_part(pvp)

            def no_fill(n):
                pass

            with tc.tile_pool(name="px", bufs=2) as px, \
                 tc.tile_pool(name="ptrig", bufs=1) as ptrig, \
                 tc.tile_pool(name="psh", bufs=2) as psh:

                # first half: plain, full PSUM for the projections
                emit_half(px, ptrig, psh, 0, 2, no_fill, nest_psum=True)

                # attention pools stay open from here through main B
                with tc.tile_pool(name="pexp", bufs=6) as pexp, \
                     tc.tile_pool(name="prr", bufs=2) as prr, \
                     tc.tile_pool(name="psc", bufs=3, space="PSUM") as psc, \
                     tc.tile_pool(name="plp", bufs=1, space="PSUM") as plp, \
                     tc.tile_pool(name="pop", bufs=1, space="PSUM") as pop:

                    for h in range(NH):
                        nc.scalar.dma_start(
                            out=wo[h], in_=woT[h * 128:(h + 1) * 128, :])

                    def b_unit_steps(h, sib):
                        """Attention unit (h, si-block): per-chunk 3-stage
                        software pipeline, yielding between stages."""
                        si0 = sib * SB
                        nch = 4 * (sib + 1)

                        def sc(c):
                            dg = c - (nch - 4)    # diagonal offset
                            lo = dg * 128 if dg > 0 else 0
                            s_ps = psc.tile([128, SB], F32, name="sps")
                            e_t = pexp.tile([128, SB], BF16, name="exp")
                            nc.tensor.matmul(
                                s_ps[:, lo:],
                                kT[h][:, c * 128:(c + 1) * 128],
                                qT[h][:, si0 + lo:si0 + SB],
                                start=True, stop=True)
                            if dg >= 0:
                                nc.vector.tensor_add(
                                    s_ps[:, lo:lo + 128],
                                    s_ps[:, lo:lo + 128], tri_t)
                            return (c, lo, s_ps, e_t)

                        def ex(stt):
                            c, lo, s_ps, e_t = stt
                            nc.scalar.activation(
                                out=e_t[:, lo:], in_=s_ps[:, lo:],
                                func=mybir.ActivationFunctionType.Exp,
                                scale=SCALE)

                        def lo_mms(stt):
                            c, lo, s_ps, e_t = stt
                            nc.tensor.matmul(
                                l_ps[:, lo:], ones_t, e_t[:, lo:],
                                start=(c == 0), stop=(c == nch - 1))
                            nc.tensor.matmul(
                                o_ps[:, lo:],
                                vn[c // 4][:, c % 4, h * 128:(h + 1) * 128],
                                e_t[:, lo:],
                                start=(c == 0), stop=(c == nch - 1))

                        l_ps = plp.tile([1, SB], F32, name="lps")
                        o_ps = pop.tile([128, SB], F32, name="ops")
                        q = deque()
                        q.append(sc(0))
                        yield
                        ex(q[0])
                        q.append(sc(1))
                        yield
                        for c in range(nch):
                            lo_mms(q.popleft())
                            if c + 1 < nch:
                                ex(q[0])
                            if c + 2 < nch:
                                q.append(sc(c + 2))
                            yield
                        recip = prr.tile([1, SB], F32, name="recip")
                        nc.vector.reciprocal_approx_fast(out=recip, in_=l_ps)
                        rb = prr.tile([128, SB], F32, name="rb")
                        nc.gpsimd.partition_broadcast(rb, recip)
                        nc.vector.tensor_mul(
                            outT[h][:, si0:si0 + SB], o_ps, rb)
                        yield

                    # second half with si-blocks 0-1 attention dribbled in
                    def b_units_01():
                        for sib in (0, 1):
                            for h in range(NH):
                                yield from b_unit_steps(h, sib)

                    bgen = b_units_01()

                    def bfill(n):
                        nonlocal bgen
                        for _ in range(n):
                            if bgen is None:
                                return
                            try:
                                next(bgen)
                            except StopIteration:
                                bgen = None
                                return

                    emit_half(px, ptrig, psh, 1, 1, bfill, nest_psum=False)
                    while bgen is not None:
                        bfill(1)

                    # ---- main attention (si-blocks 2-3) with o-projection
                    # filler for retired si-blocks ----
                    with tc.tile_pool(name="pfc", bufs=1,
                                      space="PSUM") as pfc:
                        c_queue = deque(range(8))   # st chunks of sibs 0-1
                        filler = None

                        def c_steps(st):
                            """O-projection for s-chunk st, yielding after
                            every matmul so it can be dribbled into the
                            attention pipeline."""
                            fin = pfin.tile([128, HID], BF16, name="fin")
                            for ob in range(HID // SB):
                                fps = pfc.tile([128, SB], F32, name="fps")
                                for h in range(NH):
                                    nc.tensor.matmul(
                                        fps,
                                        outT[h][:, st * 128:(st + 1) * 128],
                                        wo[h][:, ob * SB:(ob + 1) * SB],
                                        start=(h == 0), stop=(h == NH - 1))
                                    yield
                                nc.any.tensor_copy(
                                    fin[:, ob * SB:(ob + 1) * SB], fps)
                                nc.sync.dma_start(
                                    out=out[st * 128:(st + 1) * 128,
                                            ob * SB:(ob + 1) * SB],
                                    in_=fin[:, ob * SB:(ob + 1) * SB])

                        def fill(n):
                            nonlocal filler
                            for _ in range(n):
                                if filler is None:
                                    if not c_queue:
                                        return
                                    filler = c_steps(c_queue.popleft())
                                try:
                                    next(filler)
                                except StopIteration:
                                    filler = None

                        for sib in (2, 3):
                            for h in range(NH):
                                for _ in b_unit_steps(h, sib):
                                    fill(2)
                                fill(8)
                            if sib == 2:
                                c_queue.extend(range(8, 12))
                        fill(10 ** 6)

                # tail: the last si-block's four chunks; ob-major with
                # double-buffered PSUM so copies/DMAs pipeline with matmuls
                with tc.tile_pool(name="pft", bufs=4, space="PSUM") as pft:
                    for st in range(4 * (NSB - 1), S // 128):
                        fin = pfin.tile([128, HID], BF16, name="fin")
                        for ob in range(HID // SB):
                            fps = pft.tile([128, SB], F32, name="fps")
                            for h in range(NH):
                                nc.tensor.matmul(
                                    fps, outT[h][:, st * 128:(st + 1) * 128],
                                    wo[h][:, ob * SB:(ob + 1) * SB],
                                    start=(h == 0), stop=(h == NH - 1))
                            nc.any.tensor_copy(
                                fin[:, ob * SB:(ob + 1) * SB], fps)
                            nc.sync.dma_start(
                                out=out[st * 128:(st + 1) * 128,
                                        ob * SB:(ob + 1) * SB],
                                in_=fin[:, ob * SB:(ob + 1) * SB])

    nc.compile()
    return nc


_NC_CACHE = None


def _get_nc():
    global _NC_CACHE
    if _NC_CACHE is None:
        _NC_CACHE = _build_nc()
    return _NC_CACHE


def _host_inputs(x, w_qkv, w_o):
    """Per-core input maps (sharding + layout prep on host)."""
    bf = ml_dtypes.bfloat16
    inv_freq = 1.0 / (THETA ** (np.arange(0, D, 2, dtype=np.float64) / D))
    pos = np.arange(S, dtype=np.float64)
    freqs = pos[:, None] * inv_freq[None, :]          # (S, D/2)
    emb = np.concatenate([freqs, freqs], axis=-1)     # (S, D)
    cosT = np.ascontiguousarray(np.cos(emb).T.astype(bf))           # (D, S)
    sign = np.concatenate([-np.ones(D // 2), np.ones(D // 2)])
    sinST = np.ascontiguousarray((sign[None, :] * np.sin(emb)).T
                                 .astype(bf))                       # (D, S)
    # additive causal triangle for a diagonal 128x128 block of scores^T:
    # keep (add 0) when sj_local <= si_local, else -1e30
    p = np.arange(128)[:, None]
    f = np.arange(128)[None, :]
    maskadd = np.where(p <= f, 0.0, -1e30).astype(np.float32)       # (128, 128)

    xTb = [np.ascontiguousarray(x[b].T.astype(bf)) for b in range(B)]
    in_maps = []
    for c in range(NC):
        b, g = c // 4, c % 4
        rows = slice(g * NH * D, (g + 1) * NH * D)
        wq = w_qkv[0 * HID:1 * HID][rows]
        wk = w_qkv[1 * HID:2 * HID][rows]
        wv = w_qkv[2 * HID:3 * HID][rows]
        wqkvT = np.ascontiguousarray(
            np.concatenate([wq, wk, wv], axis=0).T.astype(bf))      # (HID, 1536)
        woT = np.ascontiguousarray(w_o[:, rows].T.astype(bf))       # (512, HID)
        in_maps.append({
            "xT": xTb[b], "wqkvT": wqkvT, "woT": woT,
            "cosT": cosT, "sinST": sinST, "maskadd": maskadd,
        })
    return in_maps


def kernel(x, w_qkv, w_o):
    global LAST_RESULT
    x = np.asarray(x, dtype=np.float32)
    w_qkv = np.asarray(w_qkv, dtype=np.float32)
    w_o = np.asarray(w_o, dtype=np.float32)

    nc = _get_nc()
    in_maps = _host_inputs(x, w_qkv, w_o)
    trace = bool(int(os.environ.get("BASS_KERNEL_TRACE", "0")))
    last_exc = None
    for _attempt in range(3):
        try:
            res = run_bass_kernel_spmd(
                nc, in_maps, core_ids=list(range(NC)),
                trace=trace, trace_cores=list(range(NC)) if trace else None)
            out_np = [np.asarray(res.results[c]["out"], dtype=np.float32)
                      for c in range(NC)]
            break
        except Exception as e:  # transient NRT device errors: retry
            last_exc = e
    else:
        raise last_exc
    LAST_RESULT = res

    out = np.empty((B, S, HID), dtype=np.float32)
    for b in range(B):
        acc = np.zeros((S, HID), dtype=np.float64)
        for g in range(4):
            acc += out_np[b * 4 + g]
        out[b] = acc.astype(np.float32)
    return out


# revision 38
# speedup vs baseline: 1.0226x; 1.0089x over previous
"""Causal self-attention with RoPE on 8 Trainium2 NeuronCores.

Problem: B=2, S=2048, H=16 heads, D=128, HID=2048, fp32.
  qkv = x @ w_qkv.T ; RoPE(q, k) ; causal softmax(q k^T / sqrt(D)) @ v ; out @ w_o.T

Sharding (hardcoded): core c in 0..7 handles batch b = c // 4 and head group
g = c % 4 (heads 4g..4g+4). Each core computes a partial (S, HID) output
contracted over its 512 hidden dims of the o-projection; the host sums the 4
bf16 partials per batch.

All matmul operands are bf16: fp32/fp32r LDWEIGHTS take ~220ns and serialize
with the matmuls (LDW:MM emission is 1:1), while bf16 gets Fast Weight Load
(~53-97ns, hidden) and halves SBUF/DMA/DVE cost. PSUM accumulation stays
fp32; end-to-end rel err ~3e-3 vs the 2e-2 gate.

Schedule: the engine queues are strict FIFO, so all cross-engine overlap is
arranged by emission order (software pipelining):
 - Attention for an (h, si-block) unit is a per-chunk 3-stage pipeline
   (scores matmul -> ACT exp -> l/o matmuls) with lookahead 2, so the PE
   never waits on ACT at the queue head.
 - Attention units for si-blocks 0-1 depend only on the first s-half, and
   are dribbled into the second half's projection chains (generator steps
   between projection matmuls): ACT exp runs under projection matmuls.
 - The o-projection (pure PE work) for retired si-blocks is dribbled into
   the remaining attention units' stall slots the same way.
Softmax: scores^T orientation (denominator = ones-vector matmul), no
max-subtraction (scores are O(5); exp is safe in fp32), causal masking via
an additive -1e30 triangle on diagonal 128x128 blocks + column-restricted
matmuls. The attention output o^T[d, si] is exactly the o-projection's lhsT.
"""

from collections import deque
import os

import numpy as np
import ml_dtypes

import concourse.bacc as bacc
import concourse.tile as tile
from concourse import mybir
from concourse.bass_utils import run_bass_kernel_spmd

B, S, H, D = 2, 2048, 16, 128
HID = H * D
THETA = 10000.0
SCALE = 1.0 / float(np.sqrt(D))
NH = 4                 # heads per core
NC = 8                 # cores
NKC = HID // 128       # contraction chunks for qkv projection
SB = 512               # matmul moving free dim
NSB = S // SB          # si blocks
SH = S // 2            # s-half
F32 = mybir.dt.float32
BF16 = mybir.dt.bfloat16

LAST_RESULT = None  # BassKernelResults of the most recent run (for test harness)
MM_MODE = "bf16"


def _build_nc():
    nc = bacc.Bacc("TRN2", target_bir_lowering=False, debug=False, num_devices=NC)

    xT = nc.dram_tensor("xT", [HID, S], BF16, kind="ExternalInput")
    wqkvT = nc.dram_tensor("wqkvT", [HID, 3 * NH * 128], BF16, kind="ExternalInput")
    woT = nc.dram_tensor("woT", [NH * 128, HID], BF16, kind="ExternalInput")
    cosT = nc.dram_tensor("cosT", [D, S], BF16, kind="ExternalInput")
    sinST = nc.dram_tensor("sinST", [D, S], BF16, kind="ExternalInput")
    maskadd = nc.dram_tensor("maskadd", [128, 128], F32, kind="ExternalInput")
    out = nc.dram_tensor("out", [S, HID], BF16, kind="ExternalOutput")

    with tile.TileContext(nc) as tc:
        with tc.tile_pool(name="pmisc", bufs=1) as pmisc, \
             tc.tile_pool(name="pw", bufs=1) as pw, \
             tc.tile_pool(name="pqk", bufs=1) as pqk, \
             tc.tile_pool(name="pvn", bufs=1) as pvn, \
             tc.tile_pool(name="pwo", bufs=1) as pwo, \
             tc.tile_pool(name="pfin", bufs=2) as pfin:
            ones_f32 = pmisc.tile([128, 1], F32, name="ones_f32")
            nc.vector.memset(ones_f32, 1.0)
            ones_t = pmisc.tile([128, 1], BF16, name="ones")
            nc.vector.tensor_copy(ones_t, ones_f32)
            # tri on the gpsimd queue so the scalar queue's first DMAs are
            # the weight chunks the first matmuls wait on
            tri_t = pmisc.tile([128, 128], F32, name="tri")
            nc.gpsimd.dma_start(out=tri_t, in_=maskadd[:, :])

            # warm-up: dummy matmuls keep the PE busy while the first x/w
            # DMAs land, so HAM un-throttles (K=8/8) before real work and
            # the ramp isn't paid on the first projection chain
            wsrc = pmisc.tile([128, SB], BF16, name="wsrc")
            nc.gpsimd.memset(wsrc, 0.0)
            with tc.tile_pool(name="pwarm", bufs=1, space="PSUM") as pwarm:
                warm_ps = pwarm.tile([1, SB], F32, name="warm")
                for _ in range(20):
                    nc.tensor.matmul(warm_ps, ones_t, wsrc,
                                     start=True, stop=True)

            # full qkv weights resident in SBUF, one tile per contraction
            # chunk so the first matmuls only wait on their own chunk's DMA
            w_kc = []
            for kc in range(NKC):
                wt = pw.tile([128, 3 * NH * 128], BF16, name=f"w_{kc}")
                nc.scalar.dma_start(
                    out=wt, in_=wqkvT[kc * 128:(kc + 1) * 128, :])
                w_kc.append(wt)

            qT = [pqk.tile([128, S], BF16, name=f"qT_{h}") for h in range(NH)]
            kT = [pqk.tile([128, S], BF16, name=f"kT_{h}") for h in range(NH)]
            # natural-layout v, all heads fused: vn[g][sp, j, h*128+d] covers
            # s-chunks 4g+j
            vn = [pvn.tile([128, 4, NH * 128], BF16, name=f"vn_{g}")
                  for g in range(4)]
            # attention output aliases qT: q columns for an si-block are dead
            # once that block's scores are done, and the normalized output is
            # written only after that point.
            outT = qT
            wo = [pwo.tile([128, HID], BF16, name=f"wo_{h}")
                  for h in range(NH)]

            # ---- Phase A body: q/k/v projection + RoPE for one s-half ----
            # bfill dribbles attention pipeline steps between the chains.
            def emit_half(px, ptrig, psh, half, ppa_bufs, bfill, nest_psum):
                s0 = half * SH
                xh = []
                for kc in range(NKC):
                    xt = px.tile([128, SH], BF16, name=f"xh_{kc}")
                    nc.sync.dma_start(
                        out=xt, in_=xT[kc * 128:(kc + 1) * 128, s0:s0 + SH])
                    xh.append(xt)
                cos_t = ptrig.tile([D, SH], BF16, name="cosT")
                sin_t = ptrig.tile([D, SH], BF16, name="sinST")
                nc.gpsimd.dma_start(out=cos_t, in_=cosT[:, s0:s0 + SH])
                nc.gpsimd.dma_start(out=sin_t, in_=sinST[:, s0:s0 + SH])

                def rope(h):
                    # RoPE for this head's half, in place (hides under the
                    # remaining projection matmuls)
                    for t in (qT[h], kT[h]):
                        sh_t = psh.tile([128, SH], BF16, name="shuf")
                        nc.gpsimd.dma_start(out=sh_t[0:64, :],
                                            in_=t[64:128, s0:s0 + SH])
                        nc.gpsimd.dma_start(out=sh_t[64:128, :],
                                            in_=t[0:64, s0:s0 + SH])
                        nc.vector.tensor_mul(sh_t, sh_t, sin_t)
                        nc.vector.tensor_mul(t[:, s0:s0 + SH],
                                             t[:, s0:s0 + SH], cos_t)
                        nc.vector.tensor_add(t[:, s0:s0 + SH],
                                             t[:, s0:s0 + SH], sh_t)

                def qk_head_paired(pool, h):
                    # q and k chains interleaved per weight chunk: each
                    # chunk is consumed by 4 matmuls on arrival, so the
                    # first chain isn't starved by round-robin DMA delivery
                    ps = [pool.tile([128, SB], F32, name=f"qp{i}")
                          for i in range(4)]
                    for kc in range(NKC):
                        for kind in range(2):
                            ot = kind * NH + h
                            wt = w_kc[kc][:, ot * 128:(ot + 1) * 128]
                            for sb_i in range(SH // SB):
                                nc.tensor.matmul(
                                    ps[kind * 2 + sb_i], wt,
                                    xh[kc][:, sb_i * SB:(sb_i + 1) * SB],
                                    start=(kc == 0), stop=(kc == NKC - 1))
                    for kind, dst in ((0, qT[h]), (1, kT[h])):
                        for sb_i in range(SH // SB):
                            lo = s0 + sb_i * SB
                            nc.any.tensor_copy(dst[:, lo:lo + SB],
                                               ps[kind * 2 + sb_i])
                    rope(h)

                def qk_part(ppa, heads):
                    # q/k projection, transposed output [d, s]; kc outer so
                    # the two si-blocks of one (h,kind) share a weight tile
                    for h in heads:
                        for kind, dst in ((0, qT[h]), (1, kT[h])):
                            ot = kind * NH + h
                            ps = [ppa.tile([128, SB], F32, name=f"qkps{sb}")
                                  for sb in range(SH // SB)]
                            for kc in range(NKC):
                                wt = w_kc[kc][:, ot * 128:(ot + 1) * 128]
                                for sb_i in range(SH // SB):
                                    nc.tensor.matmul(
                                        ps[sb_i], wt,
                                        xh[kc][:, sb_i * SB:(sb_i + 1) * SB],
                                        start=(kc == 0), stop=(kc == NKC - 1))
                                if kc % 3 == 2:
                                    bfill(1)
                            for sb_i in range(SH // SB):
                                lo = s0 + sb_i * SB
                                nc.any.tensor_copy(dst[:, lo:lo + SB],
                                                   ps[sb_i])
                        rope(h)

                def v_part(pvp):
                    # v projection, natural layout [s, 4 heads x d];
                    # st-outer so each chunk's PSUM->SBUF copy overlaps the
                    # next chunk's matmuls instead of serializing at the end
                    for st in range(8):
                        vps = pvp.tile([128, NH * 128], F32, name="vps")
                        for kc in range(NKC):
                            nc.tensor.matmul(
                                vps,
                                xh[kc][:, st * 128:(st + 1) * 128],
                                w_kc[kc][:, 2 * NH * 128:],
                                start=(kc == 0), stop=(kc == NKC - 1))
                            if kc % 4 == 3:
                                bfill(1)
                        sg = half * 8 + st   # global s-chunk
                        nc.any.tensor_copy(vn[sg // 4][:, sg % 4, :], vps)

                if nest_psum:
                    # first half: head 0's q/k run paired in their own
                    # scope (DMA-arrival-friendly), then distinct banks for
                    # the remaining qk and v pools so the v matmuls don't
                    # wait for the qk copies to drain the qk banks
                    with tc.tile_pool(name="ppa0", bufs=1,
                                      space="PSUM") as ppa0:
                        qk_head_paired(ppa0, 0)
                    with tc.tile_pool(name="ppa", bufs=ppa_bufs,
                                      space="PSUM") as ppa, \
                         tc.tile_pool(name="pvp", bufs=2,
                                      space="PSUM") as pvp:
                        qk_part(ppa, range(1, NH))
                        v_part(pvp)
                else:
                    with tc.tile_pool(name="ppa", bufs=ppa_bufs,
                                      space="PSUM") as ppa:
                        qk_part(ppa, range(NH))
                    with tc.tile_pool(name="pvp", bufs=2,
                                      space="PSUM") as pvp:
                        v_part(pvp)

            def no_fill(n):
                pass

            with tc.tile_pool(name="px", bufs=2) as px, \
                 tc.tile_pool(name="ptrig", bufs=1) as ptrig, \
                 tc.tile_pool(name="psh", bufs=2) as psh:

                # first half: plain, full PSUM for the projections
                emit_half(px, ptrig, psh, 0, 2, no_fill, nest_psum=True)

                # attention pools stay open from here through main B
                with tc.tile_pool(name="pexp", bufs=6) as pexp, \
                     tc.tile_pool(name="prr", bufs=2) as prr, \
                     tc.tile_pool(name="psc", bufs=3, space="PSUM") as psc, \
                     tc.tile_pool(name="plp", bufs=1, space="PSUM") as plp, \
                     tc.tile_pool(name="pop", bufs=1, space="PSUM") as pop:

                    for h in range(NH):
                        nc.scalar.dma_start(
                            out=wo[h], in_=woT[h * 128:(h + 1) * 128, :])

                    def b_unit_steps(h, sib):
                        """Attention unit (h, si-block): per-chunk 3-stage
                        software pipeline, yielding between stages."""
                        si0 = sib * SB
                        nch = 4 * (sib + 1)

                        def sc(c):
                            dg = c - (nch - 4)    # diagonal offset
                            lo = dg * 128 if dg > 0 else 0
                            s_ps = psc.tile([128, SB], F32, name="sps")
                            e_t = pexp.tile([128, SB], BF16, name="exp")
                            nc.tensor.matmul(
                                s_ps[:, lo:],
                                kT[h][:, c * 128:(c + 1) * 128],
                                qT[h][:, si0 + lo:si0 + SB],
                                start=True, stop=True)
                            if dg >= 0:
                                nc.vector.tensor_add(
                                    s_ps[:, lo:lo + 128],
                                    s_ps[:, lo:lo + 128], tri_t)
                            return (c, lo, s_ps, e_t)

                        def ex(stt):
                            c, lo, s_ps, e_t = stt
                            nc.scalar.activation(
                                out=e_t[:, lo:], in_=s_ps[:, lo:],
                                func=mybir.ActivationFunctionType.Exp,
                                scale=SCALE)

                        def lo_mms(stt):
                            c, lo, s_ps, e_t = stt
                            nc.tensor.matmul(
                                l_ps[:, lo:], ones_t, e_t[:, lo:],
                                start=(c == 0), stop=(c == nch - 1))
                            nc.tensor.matmul(
                                o_ps[:, lo:],
                                vn[c // 4][:, c % 4, h * 128:(h + 1) * 128],
                                e_t[:, lo:],
                                start=(c == 0), stop=(c == nch - 1))

                        l_ps = plp.tile([1, SB], F32, name="lps")
                        o_ps = pop.tile([128, SB], F32, name="ops")
                        q = deque()
                        q.append(sc(0))
                        yield
                        ex(q[0])
                        q.append(sc(1))
                        yield
                        for c in range(nch):
                            lo_mms(q.popleft())
                            if c + 1 < nch:
                                ex(q[0])
                            if c + 2 < nch:
                                q.append(sc(c + 2))
                            yield
                        recip = prr.tile([1, SB], F32, name="recip")
                        nc.vector.reciprocal_approx_fast(out=recip, in_=l_ps)
                        rb = prr.tile([128, SB], F32, name="rb")
                        nc.gpsimd.partition_broadcast(rb, recip)
                        nc.vector.tensor_mul(
                            outT[h][:, si0:si0 + SB], o_ps, rb)
                        yield

                    # second half with si-blocks 0-1 attention dribbled in
                    def b_units_01():
                        for sib in (0, 1):
                            for h in range(NH):
                                yield from b_unit_steps(h, sib)

                    bgen = b_units_01()

                    def bfill(n):
                        nonlocal bgen
                        for _ in range(n):
                            if bgen is None:
                                return
                            try:
                                next(bgen)
                            except StopIteration:
                                bgen = None
                                return

                    emit_half(px, ptrig, psh, 1, 1, bfill, nest_psum=False)
                    while bgen is not None:
                        bfill(1)

                    # ---- main attention (si-blocks 2-3) with o-projection
                    # filler for retired si-blocks ----
                    with tc.tile_pool(name="pfc", bufs=1,
                                      space="PSUM") as pfc:
                        c_queue = deque(range(8))   # st chunks of sibs 0-1
                        filler = None

                        def c_steps(st):
                            """O-projection for s-chunk st, yielding after
                            every matmul so it can be dribbled into the
                            attention pipeline."""
                            fin = pfin.tile([128, HID], BF16, name="fin")
                            for ob in range(HID // SB):
                                fps = pfc.tile([128, SB], F32, name="fps")
                                for h in range(NH):
                                    nc.tensor.matmul(
                                        fps,
                                        outT[h][:, st * 128:(st + 1) * 128],
                                        wo[h][:, ob * SB:(ob + 1) * SB],
                                        start=(h == 0), stop=(h == NH - 1))
                                    yield
                                nc.any.tensor_copy(
                                    fin[:, ob * SB:(ob + 1) * SB], fps)
                                nc.sync.dma_start(
                                    out=out[st * 128:(st + 1) * 128,
                                            ob * SB:(ob + 1) * SB],
                                    in_=fin[:, ob * SB:(ob + 1) * SB])

                        def fill(n):
                            nonlocal filler
                            for _ in range(n):
                                if filler is None:
                                    if not c_queue:
                                        return
                                    filler = c_steps(c_queue.popleft())
                                try:
                                    next(filler)
                                except StopIteration:
                                    filler = None

                        for sib in (2, 3):
                            for h in range(NH):
                                for _ in b_unit_steps(h, sib):
                                    fill(2 if sib == 2 else 3)
                                fill(8)
                            if sib == 2:
                                c_queue.extend(range(8, 12))
                        fill(10 ** 6)

                # tail: the last si-block's four chunks; ob-major with
                # double-buffered PSUM so copies/DMAs pipeline with matmuls
                with tc.tile_pool(name="pft", bufs=4, space="PSUM") as pft:
                    for st in range(4 * (NSB - 1), S // 128):
                        fin = pfin.tile([128, HID], BF16, name="fin")
                        for ob in range(HID // SB):
                            fps = pft.tile([128, SB], F32, name="fps")
                            for h in range(NH):
                                nc.tensor.matmul(
                                    fps, outT[h][:, st * 128:(st + 1) * 128],
                                    wo[h][:, ob * SB:(ob + 1) * SB],
                                    start=(h == 0), stop=(h == NH - 1))
                            nc.any.tensor_copy(
                                fin[:, ob * SB:(ob + 1) * SB], fps)
                            nc.sync.dma_start(
                                out=out[st * 128:(st + 1) * 128,
                                        ob * SB:(ob + 1) * SB],
                                in_=fin[:, ob * SB:(ob + 1) * SB])

    nc.compile()
    return nc


_NC_CACHE = None


def _get_nc():
    global _NC_CACHE
    if _NC_CACHE is None:
        _NC_CACHE = _build_nc()
    return _NC_CACHE


def _host_inputs(x, w_qkv, w_o):
    """Per-core input maps (sharding + layout prep on host)."""
    bf = ml_dtypes.bfloat16
    inv_freq = 1.0 / (THETA ** (np.arange(0, D, 2, dtype=np.float64) / D))
    pos = np.arange(S, dtype=np.float64)
    freqs = pos[:, None] * inv_freq[None, :]          # (S, D/2)
    emb = np.concatenate([freqs, freqs], axis=-1)     # (S, D)
    cosT = np.ascontiguousarray(np.cos(emb).T.astype(bf))           # (D, S)
    sign = np.concatenate([-np.ones(D // 2), np.ones(D // 2)])
    sinST = np.ascontiguousarray((sign[None, :] * np.sin(emb)).T
                                 .astype(bf))                       # (D, S)
    # additive causal triangle for a diagonal 128x128 block of scores^T:
    # keep (add 0) when sj_local <= si_local, else -1e30
    p = np.arange(128)[:, None]
    f = np.arange(128)[None, :]
    maskadd = np.where(p <= f, 0.0, -1e30).astype(np.float32)       # (128, 128)

    xTb = [np.ascontiguousarray(x[b].T.astype(bf)) for b in range(B)]
    in_maps = []
    for c in range(NC):
        b, g = c // 4, c % 4
        rows = slice(g * NH * D, (g + 1) * NH * D)
        wq = w_qkv[0 * HID:1 * HID][rows]
        wk = w_qkv[1 * HID:2 * HID][rows]
        wv = w_qkv[2 * HID:3 * HID][rows]
        wqkvT = np.ascontiguousarray(
            np.concatenate([wq, wk, wv], axis=0).T.astype(bf))      # (HID, 1536)
        woT = np.ascontiguousarray(w_o[:, rows].T.astype(bf))       # (512, HID)
        in_maps.append({
            "xT": xTb[b], "wqkvT": wqkvT, "woT": woT,
            "cosT": cosT, "sinST": sinST, "maskadd": maskadd,
        })
    return in_maps


def kernel(x, w_qkv, w_o):
    global LAST_RESULT
    x = np.asarray(x, dtype=np.float32)
    w_qkv = np.asarray(w_qkv, dtype=np.float32)
    w_o = np.asarray(w_o, dtype=np.float32)

    nc = _get_nc()
    in_maps = _host_inputs(x, w_qkv, w_o)
    trace = bool(int(os.environ.get("BASS_KERNEL_TRACE", "0")))
    last_exc = None
    for _attempt in range(3):
        try:
            res = run_bass_kernel_spmd(
                nc, in_maps, core_ids=list(range(NC)),
                trace=trace, trace_cores=list(range(NC)) if trace else None)
            out_np = [np.asarray(res.results[c]["out"], dtype=np.float32)
                      for c in range(NC)]
            break
        except Exception as e:  # transient NRT device errors: retry
            last_exc = e
    else:
        raise last_exc
    LAST_RESULT = res

    out = np.empty((B, S, HID), dtype=np.float32)
    for b in range(B):
        acc = np.zeros((S, HID), dtype=np.float64)
        for g in range(4):
            acc += out_np[b * 4 + g]
        out[b] = acc.astype(np.float32)
    return out
